# revision 2
# baseline (speedup 1.0000x reference)
"""Bass/Tile TRN2 kernel for nn_BasedXLLowPLinear: out = cascaded_lowp_matmul(x, w) + bias.

x: [2, 4096, 4096] f32, w: [4096, 16384] f32, bias: [16384] f32 -> out [2, 4096, 16384] f32.

Strategy: tensor-parallel over out_features across 8 cores (w/bias column-sharded,
x replicated). Per core:
  prepass: split x and the w shard into bf16 hi/lo parts in device DRAM
  main: for each N-half (w hi/lo resident in SBUF), stream x^T tiles via XBAR
        DMA-transpose and accumulate the 3-term cascade (hi*hi + lo*hi + hi*lo)
        in PSUM over the full K, add bias on eviction.
"""

import numpy as np

B, S, D_IN, D_OUT = 2, 4096, 4096, 16384
M_FULL, K_FULL = B * S, D_IN
N_CORES = 8
NSHARD = D_OUT // N_CORES
P = 128


def build_nc(M, K, NS, n_half, debug=False):
    from concourse import bacc, tile
    import concourse.mybir as mybir

    dt = mybir.dt
    KO = K // P
    N_HALVES = NS // n_half
    FREE = min(512, n_half)
    NSUB = n_half // FREE
    M_TILES = M // P

    nc = bacc.Bacc("TRN2", target_bir_lowering=False, debug=debug)
    x_d = nc.dram_tensor("x", [M, K], dt.float32, kind="ExternalInput")
    w_d = nc.dram_tensor("w", [K, NS], dt.float32, kind="ExternalInput")
    b_d = nc.dram_tensor("b", [P, NS], dt.float32, kind="ExternalInput")
    o_d = nc.dram_tensor("out", [M, NS], dt.float32, kind="ExternalOutput")

    with tile.TileContext(nc) as tc:
        with tc.tile_pool(name="dram", bufs=1, space="DRAM") as dram:
            xhi_d = dram.tile([M, K], dt.bfloat16)
            xlo_d = dram.tile([M, K], dt.bfloat16)
            whi_d = dram.tile([K, NS], dt.bfloat16)
            wlo_d = dram.tile([K, NS], dt.bfloat16)

            with tc.tile_pool(name="pre", bufs=3) as pre:
                for ko in range(KO):
                    wf = pre.tile([P, NS], dt.float32, tag="wf")
                    whi = pre.tile([P, NS], dt.bfloat16, tag="whi")
                    wlo = pre.tile([P, NS], dt.bfloat16, tag="wlo")
                    sl = slice(ko * P, (ko + 1) * P)
                    nc.sync.dma_start(wf[:], w_d[sl, :])
                    nc.gpsimd.tensor_copy(whi[:], wf[:])
                    nc.vector.tensor_sub(wlo[:], wf[:], whi[:])
                    nc.sync.dma_start(whi_d[sl, :], whi[:])
                    nc.sync.dma_start(wlo_d[sl, :], wlo[:])
                for mt in range(M_TILES):
                    xf = pre.tile([P, K], dt.float32, tag="xf")
                    hi = pre.tile([P, K], dt.bfloat16, tag="hi")
                    lo = pre.tile([P, K], dt.bfloat16, tag="lo")
                    sl = slice(mt * P, (mt + 1) * P)
                    nc.sync.dma_start(xf[:], x_d[sl, :])
                    nc.gpsimd.tensor_copy(hi[:], xf[:])
                    nc.vector.tensor_sub(lo[:], xf[:], hi[:])
                    nc.sync.dma_start(xhi_d[sl, :], hi[:])
                    nc.sync.dma_start(xlo_d[sl, :], lo[:])

            whi_v = whi_d[:].rearrange("(ko ki) n -> ki ko n", ki=P)
            wlo_v = wlo_d[:].rearrange("(ko ki) n -> ki ko n", ki=P)

            with (
                tc.tile_pool(name="const", bufs=1) as constp,
                tc.tile_pool(name="wres", bufs=1) as wres,
                tc.tile_pool(name="xtp", bufs=3) as xtp,
                tc.tile_pool(name="stag", bufs=4) as stag,
                tc.tile_pool(name="ps", bufs=2, space="PSUM") as psp,
            ):
                bias_sb = constp.tile([P, NS], dt.float32)
                nc.sync.dma_start(bias_sb[:], b_d[:])
                for h in range(N_HALVES):
                    wh = wres.tile([P, KO, n_half], dt.bfloat16, tag="wh")
                    wl = wres.tile([P, KO, n_half], dt.bfloat16, tag="wl")
                    nsl = slice(h * n_half, (h + 1) * n_half)
                    nc.sync.dma_start(wh[:], whi_v[:, :, nsl])
                    nc.sync.dma_start(wl[:], wlo_v[:, :, nsl])
                    for mt in range(M_TILES):
                        xh = xtp.tile([P, KO, P], dt.bfloat16, tag="xh")
                        xl = xtp.tile([P, KO, P], dt.bfloat16, tag="xl")
                        msl = slice(mt * P, (mt + 1) * P)
                        nc.sync.dma_start_transpose(xh[:], xhi_d[msl, :])
                        nc.sync.dma_start_transpose(xl[:], xlo_d[msl, :])
                        psums = [
                            psp.tile([P, FREE], dt.float32, tag=f"ps{i}", name=f"ps{i}")
                            for i in range(NSUB)
                        ]
                        terms = [(xh, wh), (xl, wh), (xh, wl)]
                        nk = 3 * KO
                        for t, (xt_t, wt) in enumerate(terms):
                            for ko in range(KO):
                                kk = t * KO + ko
                                for i in range(NSUB):
                                    nc.tensor.matmul(
                                        psums[i][:],
                                        xt_t[:, ko : ko + 1, :],
                                        wt[:, ko : ko + 1, i * FREE : (i + 1) * FREE],
                                        start=(kk == 0),
                                        stop=(kk == nk - 1),
                                    )
                        for i in range(NSUB):
                            ot = stag.tile([P, FREE], dt.float32, tag="ot")
                            c0 = h * n_half + i * FREE
                            nc.vector.tensor_add(
                                ot[:], psums[i][:], bias_sb[:, c0 : c0 + FREE]
                            )
                            nc.sync.dma_start(o_d[msl, c0 : c0 + FREE], ot[:])
    nc.compile()
    return nc


_NC_CACHE = {}


def _get_nc():
    key = (M_FULL, K_FULL, NSHARD)
    if key not in _NC_CACHE:
        _NC_CACHE[key] = build_nc(M_FULL, K_FULL, NSHARD, n_half=NSHARD // 2)
    return _NC_CACHE[key]


def kernel(x: np.ndarray, weight: np.ndarray, bias: np.ndarray) -> np.ndarray:
    from concourse.bass_utils import run_bass_kernel_spmd

    x2d = np.ascontiguousarray(x.reshape(M_FULL, K_FULL).astype(np.float32, copy=False))
    in_maps = []
    for c in range(N_CORES):
        nsl = slice(c * NSHARD, (c + 1) * NSHARD)
        in_maps.append(
            {
                "x": x2d,
                "w": np.ascontiguousarray(weight[:, nsl]),
                "b": np.ascontiguousarray(
                    np.broadcast_to(bias[nsl][None, :], (P, NSHARD))
                ),
            }
        )
    nc = _get_nc()
    res = run_bass_kernel_spmd(nc, in_maps, list(range(N_CORES)))
    out = np.concatenate([res.results[c]["out"] for c in range(N_CORES)], axis=1)
    return out.reshape(B, S, D_OUT)
